# revision 4
# baseline (speedup 1.0000x reference)
"""Trainium2 Bass kernel for BERT self-attention.

Problem: B=16, S=512, H=1024, 16 heads x 64. Data-parallel over batch:
each of the 8 cores owns 2 batches and runs the full attention for them.

v2 design notes (vs baseline):
  - Whole matmul datapath in bf16 (host casts X and W): 216 ns per
    512-row matmul vs 234 for fp32r, and half the HBM traffic.
  - The attention phase of the baseline ran at HALF PE clock: the PE
    p-state only ramps to max after ~3us of gap-free execution, and
    dependency stalls (EXP -> ctx) kept resetting it.  v2 software-
    pipelines the emission [scores(s) | ctx(s-1) h0 | scores'(s) |
    ctx(s-1) h1] so the PE never waits on the Scalar engine.
  - Softmax normalization commutes with the V matmul, so the kernel
    ships UNNORMALIZED ctx^T tiles [65, 512] (row 64 = denominator,
    via a ones-column in V') to the host as bf16; the host divides,
    transposes to [q, d] and adds bv.  This removes all PE transposes
    and the DVE epilogue, freeing 2 PSUM banks which the score pool
    needs for stall-free rotation (3 x 2 banks + 2 ctx banks = 8).
  - attention mask folded in as a row-scaling of V' by exp(mask/8)
    (softmax((s+m)/8) == (exp(s/8)*w) / sum(exp(s/8)*w), w=exp(m/8)).
"""

import os
import sys

import numpy as np

if "/opt/trn_rl_repo" not in sys.path:
    sys.path.insert(0, "/opt/trn_rl_repo")

NCORES = 8
B = 16
S = 512
H = 1024
NH = 16
HS = 64
B_LOC = B // NCORES          # 2 batches per core
T = B_LOC * S                # 1024 tokens per core
NK = H // 128                # 8 contraction chunks

_prog_cache = {}
last_results = None          # BassKernelResults from the most recent run


def _ensure_ntff_hook():
    """Install antenv.axon_hooks if the image lacks it (profiling only)."""
    try:
        import antenv.axon_hooks  # noqa: F401
        return
    except ImportError:
        pass
    try:
        import types
        import antenv
        from trn_agent_boot.trn_boot import _ntff_profile_via_ctypes

        mod = types.ModuleType("antenv.axon_hooks")
        state = {"hook": None}
        mod.set_axon_ntff_profile_hook = lambda h: state.__setitem__("hook", h)
        mod.get_axon_ntff_profile_hook = lambda: state["hook"]
        sys.modules["antenv.axon_hooks"] = mod
        antenv.axon_hooks = mod
        hook = _ntff_profile_via_ctypes("/opt/axon/libaxon_pjrt.so")
        if hook is not None:
            mod.set_axon_ntff_profile_hook(hook)
    except Exception as e:  # profiling is best-effort
        print(f"ntff hook install failed: {e}", file=sys.stderr)


def _build_program():
    from concourse import bacc, mybir, tile
    import concourse.bass as bass

    f32 = mybir.dt.float32
    bf = mybir.dt.bfloat16
    Exp = mybir.ActivationFunctionType.Exp
    Add = mybir.AluOpType.add
    Mult = mybir.AluOpType.mult

    nc = bacc.Bacc("TRN2", target_bir_lowering=False, debug=False,
                   enable_asserts=False)

    xT_d = nc.dram_tensor("xT", [H, T], bf, kind="ExternalInput").ap()
    wqT_d = nc.dram_tensor("wqT", [H, H], bf, kind="ExternalInput").ap()
    wkT_d = nc.dram_tensor("wkT", [H, H], bf, kind="ExternalInput").ap()
    wvT_d = nc.dram_tensor("wvT", [H, H], bf, kind="ExternalInput").ap()
    bq_d = nc.dram_tensor("bq2", [128, NK], f32, kind="ExternalInput").ap()
    bk_d = nc.dram_tensor("bk2", [128, NK], f32, kind="ExternalInput").ap()
    maskw_d = nc.dram_tensor("maskw", [128, NK], f32, kind="ExternalInput").ap()
    # out: per (b, head, [64 ctx rows + 1 denom row], q) unnormalized ctx^T
    out_d = nc.dram_tensor("out2", [B_LOC, NH, HS + 1, S], bf,
                           kind="ExternalOutput").ap()

    with tile.TileContext(nc) as tc:
        with (
            tc.tile_pool(name="const", bufs=1) as const_pool,
            tc.tile_pool(name="persist", bufs=1) as persist,
            tc.tile_pool(name="xw", bufs=1) as xw_pool,
        ):
            # constants
            bq_sb = const_pool.tile([128, NK], f32, name="bq_sb")
            bk_sb = const_pool.tile([128, NK], f32, name="bk_sb")
            maskw_sb = const_pool.tile([128, NK], f32, name="maskw_sb")
            nc.sync.dma_start(bq_sb[:], bq_d[:])
            nc.sync.dma_start(bk_sb[:], bk_d[:])
            nc.sync.dma_start(maskw_sb[:], maskw_d[:])

            # activations (transposed), kept resident; loads interleaved
            # with the first projection's weight tiles so the PE starts
            # after ~2 tiles.
            xts = [xw_pool.tile([128, T], bf, name=f"xt{k}", tag=f"xt{k}")
                   for k in range(NK)]

            qt_sb = [persist.tile([128, T], bf, name=f"qt{i}", tag=f"qt{i}")
                     for i in range(NK)]
            kt_sb = [persist.tile([128, T], bf, name=f"kt{i}", tag=f"kt{i}")
                     for i in range(NK)]
            # V' as ONE tile [128, 8 * 16 heads * 65]; col 64 of each
            # head = ones*w.  Single tile => ctx chain deps dedupe.
            vpall = persist.tile([128, NK * NH * (HS + 1)], bf, name="vpall",
                                 tag="vpall")
            vp_r = vpall.rearrange("p (t h e) -> p t h e", t=NK, e=HS + 1)

            with (
                tc.tile_pool(name="wtile", bufs=10) as w_pool,
                tc.tile_pool(name="pproj", bufs=4, space="PSUM") as pproj,
            ):
                # ---- QT / KT projections: out [o, t], bias per-partition ----
                for (pi, (w_d, dst, bias_sb)) in enumerate(
                        ((wqT_d, qt_sb, bq_sb), (wkT_d, kt_sb, bk_sb))):
                    wt = []
                    for k in range(NK):
                        w = w_pool.tile([128, H], bf, name=f"w{k}", tag="w")
                        for q in range(4):
                            r = slice(q * 32, (q + 1) * 32)
                            if pi == 0:
                                nc.sync.dma_start(
                                    xts[k][r, :], xT_d[k * 128 + q * 32:
                                                       k * 128 + q * 32 + 32, :])
                            nc.sync.dma_start(
                                w[r, :], w_d[k * 128 + q * 32:
                                             k * 128 + q * 32 + 32, :])
                        wt.append(w)
                    for ot in range(NK):
                        for th in range(2):
                            ps = pproj.tile([128, 512], f32, name="ps",
                                            tag="ps")
                            for k in range(NK):
                                nc.tensor.matmul(
                                    ps[:],
                                    wt[k][:, ot * 128:(ot + 1) * 128],
                                    xts[k][:, th * 512:(th + 1) * 512],
                                    start=(k == 0), stop=(k == NK - 1),
                                )
                            # bias add + cast on DVE (keeps Scalar free)
                            nc.vector.tensor_scalar(
                                dst[ot][:, th * 512:(th + 1) * 512], ps[:],
                                bias_sb[:, ot:ot + 1], None, Add)

                # ---- V projection: natural [t, o] into interleaved V' ----
                wt = []
                for k in range(NK):
                    w = w_pool.tile([128, H], bf, name=f"wv{k}", tag="w")
                    for q in range(4):
                        nc.sync.dma_start(
                            w[q * 32:(q + 1) * 32, :],
                            wvT_d[k * 128 + q * 32:k * 128 + q * 32 + 32, :])
                    wt.append(w)
                for tt in range(NK):
                    vv = vp_r[:, tt]
                    for oh in range(2):
                        ps = pproj.tile([128, 512], f32, name="ps", tag="ps")
                        for k in range(NK):
                            nc.tensor.matmul(
                                ps[:],
                                xts[k][:, tt * 128:(tt + 1) * 128],
                                wt[k][:, oh * 512:(oh + 1) * 512],
                                start=(k == 0), stop=(k == NK - 1),
                            )
                        # evacuate with mask scaling: V'[k,:] *= exp(m_k/8)
                        nc.vector.tensor_scalar(
                            vv[:, oh * 8:(oh + 1) * 8, 0:HS],
                            ps.rearrange("p (h d) -> p h d", d=HS),
                            maskw_sb[:, tt:tt + 1], None, Mult)
                    # ones column, scaled by mask weight (= the weight itself)
                    nc.vector.tensor_copy(
                        vv[:, :, HS:HS + 1],
                        maskw_sb[:, tt:tt + 1].broadcast_to([128, NH, 1]))

            # ---- attention: software-pipelined, ctx lags scores by 1 ----
            NSTAGE = B_LOC * (NH // 2)   # 16 (b, head-pair) stages

            with (
                tc.tile_pool(name="ex", bufs=4) as ex_pool,
                tc.tile_pool(name="cs", bufs=6) as cs_pool,
                tc.tile_pool(name="psc", bufs=3, space="PSUM") as sc_pool,
                tc.tile_pool(name="pcx", bufs=2, space="PSUM") as cx_pool,
            ):
                exs = {}      # (stage, h) -> ex tile [128, 2048]

                def emit_scores_half(s, half):
                    b, hp = divmod(s, NH // 2)
                    pair = (2 * hp, 2 * hp + 1)
                    if half == 0:
                        for h in pair:
                            exs[(s, h)] = ex_pool.tile(
                                [128, 2048], bf, name="ex", tag="ex")
                    scs = {h: sc_pool.tile([128, 1024], f32, name="sc",
                                           tag="sc")
                           for h in pair}
                    for h in pair:
                        hb = (h % 2) * HS
                        for j in range(2):
                            kt = half * 2 + j
                            c0 = b * 512 + kt * 128
                            nc.tensor.matmul(
                                scs[h][:, j * 512:(j + 1) * 512],
                                kt_sb[hp][hb:hb + HS, c0:c0 + 128],
                                qt_sb[hp][hb:hb + HS,
                                          b * 512:(b + 1) * 512],
                                start=True, stop=True,
                            )
                    for h in pair:
                        nc.scalar.activation(
                            exs[(s, h)][:, half * 1024:(half + 1) * 1024],
                            scs[h][:], Exp, scale=0.125)

                def emit_ctx(s, hi):
                    b, hp = divmod(s, NH // 2)
                    h = 2 * hp + hi
                    ex = exs[(s, h)]
                    # ctxT' = V'.T @ expT  -> [65, 512] (row 64 = denom)
                    cx = cx_pool.tile([HS + 1, 512], f32, name="cx", tag="cx")
                    for kt in range(4):
                        nc.tensor.matmul(
                            cx[:], vp_r[:, b * 4 + kt, h, :],
                            ex[:, kt * 512:(kt + 1) * 512],
                            start=(kt == 0), stop=(kt == 3),
                        )
                    cs = cs_pool.tile([HS + 1, 512], bf, name="cs", tag="cs")
                    nc.vector.tensor_copy(cs[:], cx[:])
                    nc.sync.dma_start(out_d[b, h], cs[:])
                    del exs[(s, h)]

                for s in range(NSTAGE + 1):
                    if s < NSTAGE:
                        emit_scores_half(s, 0)
                    if s >= 1:
                        emit_ctx(s - 1, 0)
                        emit_ctx(s - 1, 1)
                    if s < NSTAGE:
                        emit_scores_half(s, 1)

    nc.compile()
    return nc


def _get_program():
    if "nc" not in _prog_cache:
        _prog_cache["nc"] = _build_program()
    return _prog_cache["nc"]


def kernel(hidden_states, attention_mask, Wq, bq, Wk, bk, Wv, bv):
    global last_results
    import ml_dtypes
    from concourse import bass_utils

    bf16 = ml_dtypes.bfloat16
    hidden_states = np.ascontiguousarray(np.asarray(hidden_states,
                                                    dtype=np.float32))
    attention_mask = np.asarray(attention_mask, dtype=np.float32)
    Wq = np.asarray(Wq, dtype=np.float32)
    Wk = np.asarray(Wk, dtype=np.float32)
    Wv = np.asarray(Wv, dtype=np.float32)
    bq = np.asarray(bq, dtype=np.float32)
    bk = np.asarray(bk, dtype=np.float32)
    bv = np.asarray(bv, dtype=np.float32)

    nc = _get_program()

    wqT = np.ascontiguousarray(Wq.T.astype(bf16))
    wkT = np.ascontiguousarray(Wk.T.astype(bf16))
    wvT = np.ascontiguousarray(Wv.T.astype(bf16))
    bq2 = np.ascontiguousarray(bq.reshape(NK, 128).T)
    bk2 = np.ascontiguousarray(bk.reshape(NK, 128).T)

    mask = attention_mask.reshape(B, S)

    in_maps = []
    for c in range(NCORES):
        xT = np.ascontiguousarray(
            hidden_states[c * B_LOC:(c + 1) * B_LOC].reshape(T, H).T
            .astype(bf16))
        # maskw[p, b*4+kt] = exp(mask[b, kt*128+p] / 8)
        mw = np.exp(mask[c * B_LOC:(c + 1) * B_LOC].reshape(B_LOC, 4, 128)
                    / 8.0).transpose(2, 0, 1).reshape(128, NK)
        in_maps.append({
            "xT": xT,
            "wqT": wqT, "wkT": wkT, "wvT": wvT,
            "bq2": bq2, "bk2": bk2,
            "maskw": np.ascontiguousarray(mw.astype(np.float32)),
        })

    trace = bool(os.environ.get("BASS_TRACE"))
    if trace:
        _ensure_ntff_hook()
    res = bass_utils.run_bass_kernel_spmd(
        nc, in_maps, core_ids=list(range(NCORES)), trace=trace,
    )
    last_results = res

    # host epilogue: normalize by the denominator row, transpose to
    # [q, d], add bv (softmax normalization commutes with the V matmul).
    out = np.empty((B, S, H), dtype=np.float32)
    for c in range(NCORES):
        oc = np.asarray(res.results[c]["out2"]).astype(np.float32)
        num = oc[:, :, 0:HS, :]                    # [B_LOC, NH, 64, S]
        den = oc[:, :, HS:HS + 1, :]               # [B_LOC, NH, 1, S]
        ctx = (num / den).transpose(0, 3, 1, 2)    # [B_LOC, S, NH, 64]
        out[c * B_LOC:(c + 1) * B_LOC] = (
            ctx.reshape(B_LOC, S, H) + bv[None, None, :])
    return out


# revision 6
# speedup vs baseline: 1.3292x; 1.3292x over previous
"""Trainium2 Bass kernel for BERT self-attention.

Problem: B=16, S=512, H=1024, 16 heads x 64. Data-parallel over batch:
each of the 8 cores owns 2 batches and runs the full attention for them.

v2 design notes (vs baseline):
  - Whole matmul datapath in bf16 (host casts X and W): 216 ns per
    512-row matmul vs 234 for fp32r, and half the HBM traffic.
  - The attention phase of the baseline ran at HALF PE clock: the PE
    p-state only ramps to max after ~3us of gap-free execution, and
    dependency stalls (EXP -> ctx) kept resetting it.  v2 software-
    pipelines the emission [scores(s) | ctx(s-1) h0 | scores'(s) |
    ctx(s-1) h1] so the PE never waits on the Scalar engine.
  - Softmax normalization commutes with the V matmul, so the kernel
    ships UNNORMALIZED ctx^T tiles [65, 512] (row 64 = denominator,
    via a ones-column in V') to the host as bf16; the host divides,
    transposes to [q, d] and adds bv.  This removes all PE transposes
    and the DVE epilogue, freeing 2 PSUM banks which the score pool
    needs for stall-free rotation (3 x 2 banks + 2 ctx banks = 8).
  - attention mask folded in as a row-scaling of V' by exp(mask/8)
    (softmax((s+m)/8) == (exp(s/8)*w) / sum(exp(s/8)*w), w=exp(m/8)).
"""

import os
import sys

import numpy as np

if "/opt/trn_rl_repo" not in sys.path:
    sys.path.insert(0, "/opt/trn_rl_repo")

NCORES = 8
B = 16
S = 512
H = 1024
NH = 16
HS = 64
B_LOC = B // NCORES          # 2 batches per core
T = B_LOC * S                # 1024 tokens per core
NK = H // 128                # 8 contraction chunks

_prog_cache = {}
last_results = None          # BassKernelResults from the most recent run


def _ensure_ntff_hook():
    """Install antenv.axon_hooks if the image lacks it (profiling only)."""
    try:
        import antenv.axon_hooks  # noqa: F401
        return
    except ImportError:
        pass
    try:
        import types
        import antenv
        from trn_agent_boot.trn_boot import _ntff_profile_via_ctypes

        mod = types.ModuleType("antenv.axon_hooks")
        state = {"hook": None}
        mod.set_axon_ntff_profile_hook = lambda h: state.__setitem__("hook", h)
        mod.get_axon_ntff_profile_hook = lambda: state["hook"]
        sys.modules["antenv.axon_hooks"] = mod
        antenv.axon_hooks = mod
        hook = _ntff_profile_via_ctypes("/opt/axon/libaxon_pjrt.so")
        if hook is not None:
            mod.set_axon_ntff_profile_hook(hook)
    except Exception as e:  # profiling is best-effort
        print(f"ntff hook install failed: {e}", file=sys.stderr)


def _build_program():
    from concourse import bacc, mybir, tile
    import concourse.bass as bass

    f32 = mybir.dt.float32
    bf = mybir.dt.bfloat16
    Exp = mybir.ActivationFunctionType.Exp
    Add = mybir.AluOpType.add
    Mult = mybir.AluOpType.mult

    nc = bacc.Bacc("TRN2", target_bir_lowering=False, debug=False,
                   enable_asserts=False)

    xT_d = nc.dram_tensor("xT", [H, T], bf, kind="ExternalInput").ap()
    wqT_d = nc.dram_tensor("wqT", [H, H], bf, kind="ExternalInput").ap()
    wkT_d = nc.dram_tensor("wkT", [H, H], bf, kind="ExternalInput").ap()
    wvT_d = nc.dram_tensor("wvT", [H, H], bf, kind="ExternalInput").ap()
    bq_d = nc.dram_tensor("bq2", [128, NK], f32, kind="ExternalInput").ap()
    bk_d = nc.dram_tensor("bk2", [128, NK], f32, kind="ExternalInput").ap()
    maskw_d = nc.dram_tensor("maskw", [128, NK], f32, kind="ExternalInput").ap()
    # out: per (b, head, [64 ctx rows + 1 denom row], q) unnormalized ctx^T
    out_d = nc.dram_tensor("out2", [B_LOC, NH, HS + 1, S], bf,
                           kind="ExternalOutput").ap()

    with tile.TileContext(nc) as tc:
        with (
            tc.tile_pool(name="const", bufs=1) as const_pool,
            tc.tile_pool(name="persist", bufs=1) as persist,
            tc.tile_pool(name="xw", bufs=1) as xw_pool,
        ):
            # constants
            bq_sb = const_pool.tile([128, NK], f32, name="bq_sb")
            bk_sb = const_pool.tile([128, NK], f32, name="bk_sb")
            maskw_sb = const_pool.tile([128, NK], f32, name="maskw_sb")
            nc.sync.dma_start(bq_sb[:], bq_d[:])
            nc.sync.dma_start(bk_sb[:], bk_d[:])
            nc.sync.dma_start(maskw_sb[:], maskw_d[:])

            # activations (transposed), kept resident; loads interleaved
            # with the first projection's weight tiles so the PE starts
            # after ~2 tiles.
            xts = [xw_pool.tile([128, T], bf, name=f"xt{k}", tag=f"xt{k}")
                   for k in range(NK)]

            qt_sb = [persist.tile([128, T], bf, name=f"qt{i}", tag=f"qt{i}")
                     for i in range(NK)]
            kt_sb = [persist.tile([128, T], bf, name=f"kt{i}", tag=f"kt{i}")
                     for i in range(NK)]
            # V' as ONE tile [128, 8 * 16 heads * 65]; col 64 of each
            # head = ones*w.  Single tile => ctx chain deps dedupe.
            vpall = persist.tile([128, NK * NH * (HS + 1)], bf, name="vpall",
                                 tag="vpall")
            vp_r = vpall.rearrange("p (t h e) -> p t h e", t=NK, e=HS + 1)

            with (
                tc.tile_pool(name="wtile", bufs=18) as w_pool,
                tc.tile_pool(name="pproj", bufs=4, space="PSUM") as pproj,
            ):
                # ---- QT / KT projections: out [o, t], bias per-partition ----
                for (pi, (w_d, dst, bias_sb)) in enumerate(
                        ((wqT_d, qt_sb, bq_sb), (wkT_d, kt_sb, bk_sb))):
                    wt = []
                    for k in range(NK):
                        w = w_pool.tile([128, H], bf, name=f"w{k}", tag="w")
                        if pi == 0 and k == 0:
                            # first tiles gate the PE start: halve for
                            # 2-queue parallelism (bounded dep fanin)
                            nc.sync.dma_start(xts[0][0:64, :], xT_d[0:64, :])
                            nc.sync.dma_start(xts[0][64:128, :],
                                              xT_d[64:128, :])
                            nc.sync.dma_start(w[0:64, :], w_d[0:64, :])
                            nc.sync.dma_start(w[64:128, :], w_d[64:128, :])
                        else:
                            if pi == 0:
                                nc.sync.dma_start(xts[k][:],
                                                  xT_d[k * 128:(k + 1) * 128,
                                                       :])
                            nc.sync.dma_start(w[:],
                                              w_d[k * 128:(k + 1) * 128, :])
                        wt.append(w)
                    for ot in range(NK):
                        for th in range(2):
                            ps = pproj.tile([128, 512], f32, name="ps",
                                            tag="ps")
                            for k in range(NK):
                                nc.tensor.matmul(
                                    ps[:],
                                    wt[k][:, ot * 128:(ot + 1) * 128],
                                    xts[k][:, th * 512:(th + 1) * 512],
                                    start=(k == 0), stop=(k == NK - 1),
                                )
                            # bias add + cast on DVE (keeps Scalar free)
                            nc.vector.tensor_scalar(
                                dst[ot][:, th * 512:(th + 1) * 512], ps[:],
                                bias_sb[:, ot:ot + 1], None, Add)

                # ---- V projection: natural [t, o] into interleaved V' ----
                wt = []
                for k in range(NK):
                    w = w_pool.tile([128, H], bf, name=f"wv{k}", tag="w")
                    nc.sync.dma_start(w[:], wvT_d[k * 128:(k + 1) * 128, :])
                    wt.append(w)
                for tt in range(NK):
                    vv = vp_r[:, tt]
                    for oh in range(2):
                        ps = pproj.tile([128, 512], f32, name="ps", tag="ps")
                        for k in range(NK):
                            nc.tensor.matmul(
                                ps[:],
                                xts[k][:, tt * 128:(tt + 1) * 128],
                                wt[k][:, oh * 512:(oh + 1) * 512],
                                start=(k == 0), stop=(k == NK - 1),
                            )
                        # evacuate with mask scaling: V'[k,:] *= exp(m_k/8)
                        nc.vector.tensor_scalar(
                            vv[:, oh * 8:(oh + 1) * 8, 0:HS],
                            ps.rearrange("p (h d) -> p h d", d=HS),
                            maskw_sb[:, tt:tt + 1], None, Mult)
                    # ones column, scaled by mask weight (= the weight itself)
                    nc.vector.tensor_copy(
                        vv[:, :, HS:HS + 1],
                        maskw_sb[:, tt:tt + 1].broadcast_to([128, NH, 1]))

            # ---- attention: software-pipelined, ctx lags scores by 1 ----
            NSTAGE = B_LOC * (NH // 2)   # 16 (b, head-pair) stages

            with (
                tc.tile_pool(name="ex", bufs=4) as ex_pool,
                tc.tile_pool(name="cs", bufs=6) as cs_pool,
                tc.tile_pool(name="psc", bufs=3, space="PSUM") as sc_pool,
                tc.tile_pool(name="pcx", bufs=2, space="PSUM") as cx_pool,
            ):
                exs = {}      # (stage, h) -> ex tile [128, 2048]

                def emit_scores_half(s, half):
                    b, hp = divmod(s, NH // 2)
                    pair = (2 * hp, 2 * hp + 1)
                    if half == 0:
                        for h in pair:
                            exs[(s, h)] = ex_pool.tile(
                                [128, 2048], bf, name="ex", tag="ex")
                    scs = {h: sc_pool.tile([128, 1024], f32, name="sc",
                                           tag="sc")
                           for h in pair}
                    for h in pair:
                        hb = (h % 2) * HS
                        for j in range(2):
                            kt = half * 2 + j
                            c0 = b * 512 + kt * 128
                            nc.tensor.matmul(
                                scs[h][:, j * 512:(j + 1) * 512],
                                kt_sb[hp][hb:hb + HS, c0:c0 + 128],
                                qt_sb[hp][hb:hb + HS,
                                          b * 512:(b + 1) * 512],
                                start=True, stop=True,
                            )
                    for h in pair:
                        nc.scalar.activation(
                            exs[(s, h)][:, half * 1024:(half + 1) * 1024],
                            scs[h][:], Exp, scale=0.125)

                def emit_ctx(s, hi):
                    b, hp = divmod(s, NH // 2)
                    h = 2 * hp + hi
                    ex = exs[(s, h)]
                    # ctxT' = V'.T @ expT  -> [65, 512] (row 64 = denom)
                    cx = cx_pool.tile([HS + 1, 512], f32, name="cx", tag="cx")
                    for kt in range(4):
                        nc.tensor.matmul(
                            cx[:], vp_r[:, b * 4 + kt, h, :],
                            ex[:, kt * 512:(kt + 1) * 512],
                            start=(kt == 0), stop=(kt == 3),
                        )
                    cs = cs_pool.tile([HS + 1, 512], bf, name="cs", tag="cs")
                    nc.vector.tensor_copy(cs[:], cx[:])
                    nc.sync.dma_start(out_d[b, h], cs[:])
                    del exs[(s, h)]

                for s in range(NSTAGE + 1):
                    if s < NSTAGE:
                        emit_scores_half(s, 0)
                    if s >= 1:
                        emit_ctx(s - 1, 0)
                        emit_ctx(s - 1, 1)
                    if s < NSTAGE:
                        emit_scores_half(s, 1)

    nc.compile()
    return nc


def _get_program():
    if "nc" not in _prog_cache:
        _prog_cache["nc"] = _build_program()
    return _prog_cache["nc"]


def kernel(hidden_states, attention_mask, Wq, bq, Wk, bk, Wv, bv):
    global last_results
    import ml_dtypes
    from concourse import bass_utils

    bf16 = ml_dtypes.bfloat16
    hidden_states = np.ascontiguousarray(np.asarray(hidden_states,
                                                    dtype=np.float32))
    attention_mask = np.asarray(attention_mask, dtype=np.float32)
    Wq = np.asarray(Wq, dtype=np.float32)
    Wk = np.asarray(Wk, dtype=np.float32)
    Wv = np.asarray(Wv, dtype=np.float32)
    bq = np.asarray(bq, dtype=np.float32)
    bk = np.asarray(bk, dtype=np.float32)
    bv = np.asarray(bv, dtype=np.float32)

    nc = _get_program()

    wqT = np.ascontiguousarray(Wq.T.astype(bf16))
    wkT = np.ascontiguousarray(Wk.T.astype(bf16))
    wvT = np.ascontiguousarray(Wv.T.astype(bf16))
    bq2 = np.ascontiguousarray(bq.reshape(NK, 128).T)
    bk2 = np.ascontiguousarray(bk.reshape(NK, 128).T)

    mask = attention_mask.reshape(B, S)

    in_maps = []
    for c in range(NCORES):
        xT = np.ascontiguousarray(
            hidden_states[c * B_LOC:(c + 1) * B_LOC].reshape(T, H).T
            .astype(bf16))
        # maskw[p, b*4+kt] = exp(mask[b, kt*128+p] / 8)
        mw = np.exp(mask[c * B_LOC:(c + 1) * B_LOC].reshape(B_LOC, 4, 128)
                    / 8.0).transpose(2, 0, 1).reshape(128, NK)
        in_maps.append({
            "xT": xT,
            "wqT": wqT, "wkT": wkT, "wvT": wvT,
            "bq2": bq2, "bk2": bk2,
            "maskw": np.ascontiguousarray(mw.astype(np.float32)),
        })

    trace = bool(os.environ.get("BASS_TRACE"))
    if trace:
        _ensure_ntff_hook()
    res = bass_utils.run_bass_kernel_spmd(
        nc, in_maps, core_ids=list(range(NCORES)), trace=trace,
    )
    last_results = res

    # host epilogue: normalize by the denominator row, transpose to
    # [q, d], add bv (softmax normalization commutes with the V matmul).
    out = np.empty((B, S, H), dtype=np.float32)
    for c in range(NCORES):
        oc = np.asarray(res.results[c]["out2"]).astype(np.float32)
        num = oc[:, :, 0:HS, :]                    # [B_LOC, NH, 64, S]
        den = oc[:, :, HS:HS + 1, :]               # [B_LOC, NH, 1, S]
        ctx = (num / den).transpose(0, 3, 1, 2)    # [B_LOC, S, NH, 64]
        out[c * B_LOC:(c + 1) * B_LOC] = (
            ctx.reshape(B_LOC, S, H) + bv[None, None, :])
    return out
